# revision 26
# baseline (speedup 1.0000x reference)
"""Multi-headed attention (B=4, S=2048, D=512, H=8) on 8 TRN2 NeuronCores.

Sharding: core c handles batch b = c//2 and head-group hg = c%2 (4 of the 8
heads, i.e. a 256-wide slice of the model dim). Each core computes the full
attention for its (batch, 4 heads) and a partial output projection through
the matching 256-column slice of Wo. The host sums the two partials per
batch and adds the (output + folded V) bias.

Exact algebraic simplifications vs the reference:
  - K bias dropped: scores with/without bk differ by a per-query constant,
    which cancels in softmax.
  - V bias folded out: softmax rows sum to 1, so attn(V+bv) = attn(V) + bv;
    the bv @ Wo^T term is added to the output bias on the host.

Per-core kernel (matmul operands float32r):
  QT/KT [256, 2048] = W{q,k}T.T @ XT   (e on partitions, seq on free dim)
  V     [2048, 256] = XT.T @ WvT       (+ a ones column per head)
  per (head-pair p, q-block 512) - a software pipeline over 16 k-strips:
      scoresT [128, 1024] psum: two K=64 matmuls at base partitions 0/64
          (the PE packs them into disjoint array halves, so the pair
          streams in ~one matmul's time)
      exp: one ACTIVATE over the [128, 1024] psum tile -> SBUF (f32r)
      attnV, lagging 3 strips behind exp: per head, [65, 512] psum
          accumulation (the lag keeps PE weight loads prefetched and
          absorbs the Act engine's latency; the last 3 strips of a pair
          spill into the next pair's first strip)
      normalize: denom row -> K=1 matmul broadcast to 64 partitions ->
          fast reciprocal -> DVE multiply -> at (head pairs packed on
          128 partitions for the y projection)
  y [2048, 512] partial: per s-tile, 2 accumulating K=128 matmuls
      (head-pair packed) -> DVE copy -> DMA out.

Projection / y-tile chains ride inside the strip loops as "hooks" so
their matmuls fill PE slack in the Act-bound strip pipeline; X arrives
in 512-seq-column DMA chunks consumed just-in-time by those chains.
"""

import numpy as np

S = 2048          # sequence length
D = 512           # model dim
EL = 256          # local (per-core) slice of model dim = 4 heads * 64
H = 4             # local heads
DH = 64           # head dim
P = 128           # partitions
NKC = D // P      # k chunks for projections (4)
NST = S // P      # s tiles of 128 (16)
NQB = S // 512    # q blocks of 512 (4)

_CACHE = {}


def _build_nc(pt_bufs=8):
    import concourse.bacc as bacc
    import concourse.mybir as mybir
    import concourse.tile as tile

    F32 = mybir.dt.float32
    F32R = mybir.dt.float32r
    EXP = mybir.ActivationFunctionType.Exp

    nc = bacc.Bacc()

    XT = nc.declare_dram_parameter("XT", [D, S], F32R, isOutput=False)
    WQT = nc.declare_dram_parameter("WQT", [D, EL], F32R, isOutput=False)
    WKT = nc.declare_dram_parameter("WKT", [D, EL], F32R, isOutput=False)
    WVT = nc.declare_dram_parameter("WVT", [D, EL], F32R, isOutput=False)
    WOT = nc.declare_dram_parameter("WOT", [EL, D], F32R, isOutput=False)
    BQ = nc.declare_dram_parameter("BQ", [EL], F32, isOutput=False)
    Y = nc.declare_dram_parameter("Y", [S, D], F32, isOutput=True)

    with tile.TileContext(nc) as tc:
        with (
            tc.tile_pool(name="persist", bufs=1) as pp,
            tc.tile_pool(name="pt", bufs=pt_bufs) as ptp,
            tc.tile_pool(name="dn", bufs=2) as dnp,
            tc.tile_pool(name="rb", bufs=2) as rbp,
            tc.tile_pool(name="ysb", bufs=3) as ysbp,
            tc.tile_pool(name="ps", bufs=2, space="PSUM") as psp,
        ):
            # ---- persistent SBUF tiles ----
            xt = pp.tile([P, NKC, S], F32R)          # X^T, d on partitions
            wqt = pp.tile([P, NKC, EL], F32R)
            wkt = pp.tile([P, NKC, EL], F32R)
            wvt = pp.tile([P, NKC, EL], F32R)
            wot = pp.tile([P, 2, D], F32R)           # head-pair g packed
            bq = pp.tile([P, 2], F32)
            qt = pp.tile([P, 2, S], F32R)            # Q^T (e-chunk on outer)
            kt = pp.tile([P, 2, S], F32R)
            vv = pp.tile([P, NST, H, DH + 1], F32R)  # V + ones col per head
            at = pp.tile([P, 2, S], F32R)            # normalized attn^T

            # ---- loads: small weights first, X^T per-512-seq chunks so the
            # first projections + attention start after ~1MB ----
            xt_src = XT.rearrange("(o p) s -> p o s", p=P)

            def load_x(c0, c1):
                nc.sync.dma_start(
                    xt[:, :, c0:c1], xt_src[:, :, c0:c1]
                )

            # first-needed first: the K/Q chains for pair 0 only need wkt/wqt
            # + X chunk 0; V projections start a few strips in (wvt); later X
            # chunks are consumed JIT by hooked chains.
            nc.sync.dma_start(wkt[:], WKT.rearrange("(o p) e -> p o e", p=P))
            nc.sync.dma_start(wqt[:], WQT.rearrange("(o p) e -> p o e", p=P))
            nc.sync.dma_start(bq[:], BQ.rearrange("(o p) -> p o", p=P))
            # wvt goes right after X chunk 0 so pair 0's V projections
            # aren't starved behind 3MB of X.
            load_x(0, 512)
            nc.sync.dma_start(wvt[:], WVT.rearrange("(o p) e -> p o e", p=P))
            for sb in range(1, 4):
                load_x(sb * 512, (sb + 1) * 512)
            nc.sync.dma_start(wot[:], WOT.rearrange("(g p) e -> p g e", p=P))

            # fill V tile with 1.0 (broadcast copy; memset can't write f32r);
            # V-proj copies overwrite cols 0:DH of each head, leaving col DH
            # as the ones column that produces the softmax denominator
            ones = pp.tile([P, 1, 1, 1], F32)
            nc.vector.memset(ones[:], 1.0)
            nc.vector.tensor_copy(vv[:], ones[:].to_broadcast([P, NST, H, DH + 1]))
            # ones row, lhsT of the K=1 broadcast matmul in the normalize step
            ones2 = pp.tile([1, DH], F32R)
            nc.vector.tensor_copy(ones2[:], ones[0:1, 0, 0, :].to_broadcast([1, DH]))

            # dummy matmuls during the DMA-bound lead-in: the PE clock needs
            # ~3us of busy time to ramp to full speed, so burn the DMA wait
            # keeping it hot instead of ramping inside the first real chains
            wz = pp.tile([1, 512], F32R)
            nc.vector.tensor_copy(wz[:], ones[0:1, 0, 0, :].to_broadcast([1, 512]))
            warm = psp.tile([DH, 512], F32, tag="yp")
            for _ in range(20):
                nc.tensor.matmul(warm[:], ones2[:], wz[:], skip_group_check=True)

            # ---- emission helpers ----
            def proj_k(e, sb, c0=0, w=512):
                s0 = sb * 512 + c0
                ps = psp.tile([P, 512], F32, tag="yp", name="ps")
                for k in range(NKC):
                    nc.tensor.matmul(
                        ps[:, :w],
                        wkt[:, k, e * P : (e + 1) * P],
                        xt[:, k, s0 : s0 + w],
                        start=(k == 0),
                        stop=(k == NKC - 1),
                        skip_group_check=True,
                    )
                nc.vector.tensor_copy(kt[:, e, s0 : s0 + w], ps[:, :w])

            def proj_q(e, sb, c0=0, w=512):
                s0 = sb * 512 + c0
                ps = psp.tile([P, 512], F32, tag="yp", name="ps")
                for k in range(NKC):
                    nc.tensor.matmul(
                        ps[:, :w],
                        wqt[:, k, e * P : (e + 1) * P],
                        xt[:, k, s0 : s0 + w],
                        start=(k == 0),
                        stop=(k == NKC - 1),
                        skip_group_check=True,
                    )
                nc.vector.tensor_scalar_add(
                    qt[:, e, s0 : s0 + w], ps[:, :w], bq[:, e : e + 1]
                )

            def proj_v(st):
                ps = psp.tile([P, 512], F32, tag="yp")
                for k in range(NKC):
                    nc.tensor.matmul(
                        ps[:, :EL],
                        xt[:, k, st * P : (st + 1) * P],
                        wvt[:, k, :],
                        start=(k == 0),
                        stop=(k == NKC - 1),
                        skip_group_check=True,
                    )
                nc.vector.tensor_copy(
                    vv[:, st, :, 0:DH],
                    ps[:, :EL].rearrange("p (h d) -> p h d", h=H),
                )

            def ytile(st):
                yp = psp.tile([P, 512], F32, tag="yp")
                for g in range(2):
                    nc.tensor.matmul(
                        yp[:],
                        at[:, g, st * P : (st + 1) * P],
                        wot[:, g, :],
                        start=(g == 0),
                        stop=(g == 1),
                        skip_group_check=True,
                    )
                ysb = ysbp.tile([P, 512], F32, tag="ysb")
                nc.vector.tensor_copy(ysb[:], yp[:])
                nc.sync.dma_start(Y[st * P : (st + 1) * P, :], ysb[:])

            class Pair:
                """State for one (q-block, head-pair) attention pipeline."""

                def __init__(self, qb, p):
                    self.qb, self.p = qb, p
                    self.avA = psp.tile([DH + 1, 512], F32, tag="av")
                    self.avB = psp.tile([DH + 1, 512], F32, tag="av")
                    self.pts = [None] * NST

                def scores(self, ks):
                    q0 = self.qb * 512
                    sc = psp.tile([P, 1024], F32, tag="sc")
                    k0 = ks * P
                    nc.tensor.matmul(
                        sc[:, 0:512],
                        kt[0:DH, self.p, k0 : k0 + P],
                        qt[0:DH, self.p, q0 : q0 + 512],
                    )
                    nc.tensor.matmul(
                        sc[:, 512:1024],
                        kt[DH:P, self.p, k0 : k0 + P],
                        qt[DH:P, self.p, q0 : q0 + 512],
                    )
                    pt = ptp.tile([P, 1024], F32R, tag="pt")
                    nc.scalar.activation(pt[:], sc[:], EXP)
                    self.pts[ks] = pt

                def attnv(self, ks):
                    for hp, av in ((0, self.avA), (1, self.avB)):
                        h = 2 * self.p + hp
                        nc.tensor.matmul(
                            av[:],
                            vv[:, ks, h, :],
                            self.pts[ks][:, hp * 512 : (hp + 1) * 512],
                            start=(ks == 0),
                            stop=(ks == NST - 1),
                            skip_group_check=True,
                        )
                    self.pts[ks] = None

                def trans(self):
                    # normalize: denom row -> K=1 broadcast matmul to 64
                    # partitions -> fast recip -> DVE multiply into packed at
                    q0 = self.qb * 512
                    for hp, av in ((0, self.avA), (1, self.avB)):
                        dn = dnp.tile([1, 512], F32R, tag="dn")
                        nc.vector.tensor_copy(dn[:], av[DH : DH + 1, :])
                        rb_ps = psp.tile([DH, 512], F32, tag="yp")
                        nc.tensor.matmul(rb_ps[:], ones2[:], dn[:])
                        rb = rbp.tile([DH, 512], F32, tag="rb")
                        nc.vector.reciprocal_approx_fast(rb[:], rb_ps[:])
                        nc.vector.tensor_mul(
                            at[hp * DH : (hp + 1) * DH, self.p, q0 : q0 + 512],
                            av[0:DH, :],
                            rb[:],
                        )

            # ---- schedule ----
            # Global software pipeline over 8 pairs x 16 strips. Per strip:
            # scores+exp first (keeps Act saturated), then attnV per the lag
            # schedule (leftover strips of the previous pair spill into the
            # next pair's first strips, normalize right after, and the new
            # pair's accumulation starts at strip 3 once its PSUM slot is
            # free), then hook chains (projections / y tiles) fill PE slack.
            #
            # lead-in: K^T e0 block 0 + Q^T e0 block 0 only (first X chunk);
            # the rest of K^T e0 is hooked JIT as later X chunks land.
            # lead-in: K^T e-chunk 0 fully + Q^T e0 block 0 (pair (0,0)
            # needs them); everything else rides inside strip-loop hooks.
            proj_k(0, 0)
            proj_q(0, 0)
            proj_k(0, 1)
            proj_k(0, 2)
            proj_k(0, 3)

            hooks = {J: {} for J in range(8)}

            def hook(J, ks, th):
                hooks[J].setdefault(ks, []).append(th)

            # pair 0 = (0,0): V tiles JIT (attnv(ks) runs at strip ks+3),
            # then pair 1's first chains.
            for ks in range(NST):
                hook(0, ks, lambda st=ks: proj_v(st))
            hook(0, 4, lambda: proj_k(1, 0))
            hook(0, 9, lambda: proj_q(1, 0))
            # pair 1 = (0,1): remaining K^T e1 blocks JIT (block b needed by
            # strip 4b), Q^T e0 block 1 for pair 2.
            hook(1, 1, lambda: proj_k(1, 1))
            hook(1, 5, lambda: proj_k(1, 2))
            hook(1, 9, lambda: proj_k(1, 3))
            hook(1, 12, lambda: proj_q(0, 1))
            # later pairs: y tiles of q-block qb-1 + Q^T blocks JIT.
            qproj = {2: (1, 1), 3: (0, 2), 4: (1, 2), 5: (0, 3), 6: (1, 3)}
            for J in range(2, 8):
                st0 = (J // 2 - 1) * 4 + (J % 2) * 2
                hook(J, 5, lambda st=st0: ytile(st))
                hook(J, 9, lambda st=st0 + 1: ytile(st))
                if J in qproj:
                    hook(J, 12, lambda eb=qproj[J]: proj_q(*eb))

            prev = None
            for J in range(8):
                cur = Pair(J // 2, J % 2)
                for ks in range(NST):
                    cur.scores(ks)
                    if ks == 0 and prev is not None:
                        prev.attnv(13)
                        prev.attnv(14)
                        prev.attnv(15)
                        prev.trans()
                    elif ks >= 3:
                        cur.attnv(ks - 3)
                    for th in hooks[J].get(ks, ()):
                        th()
                prev = cur
            for ks in range(13, NST):
                prev.attnv(ks)
            prev.trans()
            for st in range((NQB - 1) * 4, NQB * 4):
                ytile(st)

    nc.finalize()
    return nc


def _get_nc():
    if "nc" not in _CACHE:
        _CACHE["nc"] = _build_nc()
    return _CACHE["nc"]


def _prep_inputs(X, Wq, bq, Wk, bk, Wv, bv, Wo, bo):
    f = lambda a: np.ascontiguousarray(np.asarray(a), dtype=np.float32)
    X, Wq, bq, Wk, bk, Wv, bv, Wo, bo = map(f, (X, Wq, bq, Wk, bk, Wv, bv, Wo, bo))
    B = X.shape[0]
    scale = np.float32(1.0 / np.sqrt(DH))
    XT = [np.ascontiguousarray(X[b].T) for b in range(B)]
    in_maps = []
    for c in range(2 * B):
        b, hg = divmod(c, 2)
        sl = slice(hg * EL, (hg + 1) * EL)
        in_maps.append(
            {
                "XT": XT[b],
                "WQT": np.ascontiguousarray((Wq[sl] * scale).T),
                "WKT": np.ascontiguousarray(Wk[sl].T),
                "WVT": np.ascontiguousarray(Wv[sl].T),
                "WOT": np.ascontiguousarray(Wo[:, sl].T),
                "BQ": np.ascontiguousarray(bq[sl] * scale),
            }
        )
    # V bias folded through softmax into the output bias
    bo_eff = (bo + bv @ Wo.T).astype(np.float32)
    return in_maps, bo_eff, B


def run(inputs, trace=False, trace_cores=None):
    """Run the kernel; returns (Y_full, exec_time_ns or None)."""
    from concourse.bass_utils import run_bass_kernel_spmd

    in_maps, bo_eff, B = _prep_inputs(**inputs)
    nc = _get_nc()
    kw = {}
    if trace:
        kw = dict(trace=True, trace_cores=trace_cores or list(range(2 * B)))
    res = run_bass_kernel_spmd(nc, in_maps, list(range(2 * B)), **kw)
    Y = np.stack(
        [
            res.results[2 * b]["Y"] + res.results[2 * b + 1]["Y"] + bo_eff
            for b in range(B)
        ]
    )
    return Y, getattr(res, "exec_time_ns", None)


def kernel(X, Wq, bq, Wk, bk, Wv, bv, Wo, bo):
    Y, _ = run(
        dict(X=X, Wq=Wq, bq=bq, Wk=Wk, bk=bk, Wv=Wv, bv=bv, Wo=Wo, bo=bo)
    )
    return Y
